# revision 16
# baseline (speedup 1.0000x reference)
"""Trainium2 Bass kernel for mutual-nearest-neighbor matching (Lowe ratio test).

Per-core layout: batch b=8 is sharded 1 batch element per NeuronCore (8 cores).
Each core computes, for its batch element:
  sim = d0^T @ d1          [n=4096, m=4096]   (bf16 matmuls, fp32 PSUM accum)
  top-2 + argmax along m  -> matches0 candidates + ratio mask + scores
  sim^T = d1^T @ d0        (second matmul direction)
  top-2 + argmax along n  -> matches1 candidates + ratio mask
  mutual check (fully local, via one small wrapped gather)
Outputs: matches int32 [4096], scores f32 [4096] per core; host stacks to [8, 4096].

v3 pipeline per 128-row tile:
  ACT evicts each PSUM half fp32 -> fp16 SBUF (exact to fp16, no bit tricks).
  DVE folds X [128, 4096] -> F4 [128, 256] with 4 levels of *within-16-block*
  pairwise max (block-local fan-in via 3D access patterns keeps step-1 inner
  runs, so levels 1-3 run in the packed 2x DVE mode).  F4[g] = max of block
  X[16g:16g+16).  Max8 gives the top-8 (so v1 and the cross-block runner-up
  v2), MaxIndex gives the winning block pf.  A per-tile gpsimd indirect_copy
  gathers the winner's 16-column block; a diag mask + reduce extracts each
  row's own block, and the epilogue recovers the exact column as
  16*pf + (first position equal to v1).
  No mantissa truncation anywhere: values are exact fp16, v2 is exact unless
  the top-2 share one 16-column block (then the ratio test sees the best
  value outside the block -- indistinguishable for any realistic margin).
Direction-1 epilogues run in chunks; each chunk's matches1 slice bounces to
DRAM (r-major, contiguous) and is broadcast to all partitions by a stride-0
DMA read while later tiles still compute.  The mutual check then needs one
small wrapped gather (indices permuted into the r-major layout).
"""

import sys

if "/opt/trn_rl_repo" not in sys.path:
    sys.path.insert(0, "/opt/trn_rl_repo")

import numpy as np
import ml_dtypes

B, D, N, M = 8, 256, 4096, 4096
NT = N // 128            # 32 row tiles per direction
HALF = M // 2            # 2048 columns per PSUM half-tile
NBANK = HALF // 512      # 4 matmul banks per half-tile
NSLOT = NT               # 32 row-tile slots per direction
RATIO2 = 0.8 * 0.8       # Lowe ratio threshold squared

FW4 = M // 16            # 256: width of the final fold array F4
NCHUNK = 4               # dir-1 epilogue chunks

_CACHE: dict = {}


def _build_program(debug=False):
    import concourse.mybir as mybir
    import concourse.tile as tile
    from concourse import bacc

    dt = mybir.dt
    Alu = mybir.AluOpType

    nc = bacc.Bacc("TRN2", target_bir_lowering=False, debug=False)

    d0_dram = nc.dram_tensor("d0", [2, 128, N], dt.bfloat16, kind="ExternalInput")
    d1_dram = nc.dram_tensor("d1", [2, 128, M], dt.bfloat16, kind="ExternalInput")
    matches_dram = nc.dram_tensor("matches", [N], dt.int32, kind="ExternalOutput")
    scores_dram = nc.dram_tensor("scores", [N], dt.float32, kind="ExternalOutput")
    m1_bounce = nc.dram_tensor("m1_bounce", [M], dt.float32)  # internal
    c_indsn_dram = nc.dram_tensor("c_indsn", [128, NT], dt.float32, kind="ExternalInput")
    c_diagf_dram = nc.dram_tensor("c_diagf", [128, 16 * NT], dt.float32, kind="ExternalInput")
    c_iota16_dram = nc.dram_tensor("c_iota16", [128, 16], dt.uint16, kind="ExternalInput")
    c_diag256_dram = nc.dram_tensor("c_diag256", [128, 256], dt.float16, kind="ExternalInput")
    c_irev_dram = nc.dram_tensor("c_irev", [128, 16 * NT], dt.float16, kind="ExternalInput")
    if debug:
        dbg_m0 = nc.dram_tensor("dbg_m0", [N], dt.float32, kind="ExternalOutput")
        dbg_m1 = nc.dram_tensor("dbg_m1", [M], dt.float32, kind="ExternalOutput")
        dbg_loop = nc.dram_tensor("dbg_loop", [N], dt.float32, kind="ExternalOutput")
        dbg_inds = nc.dram_tensor("dbg_inds", [N], dt.float32, kind="ExternalOutput")

    with tile.TileContext(nc) as tc:
        with (
            tc.tile_pool(name="w", bufs=1) as wpool,
            tc.tile_pool(name="consts", bufs=1) as cpool,
            tc.tile_pool(name="acc", bufs=1) as apool,
            tc.tile_pool(name="x", bufs=4) as xpool,
            tc.tile_pool(name="f", bufs=3) as fpool,
            tc.tile_pool(name="psum", bufs=2, space="PSUM") as ppool,
        ):
            # ---- load descriptors (already bf16, k-major [2, 128, N]) ----
            # d1 first (tile-0 matmuls need all of d1); d0 split so its first
            # columns arrive early.
            d0_sb = [wpool.tile([128, N], dt.bfloat16, name=f"d0_{k}") for k in range(2)]
            d1_sb = [wpool.tile([128, M], dt.bfloat16, name=f"d1_{k}") for k in range(2)]
            for k in range(2):
                nc.sync.dma_start(d1_sb[k][:], d1_dram[k])
            D0SPLIT = 512
            for k in range(2):
                nc.sync.dma_start(d0_sb[k][:, :D0SPLIT], d0_dram[k][:, :D0SPLIT])
            for k in range(2):
                nc.sync.dma_start(d0_sb[k][:, D0SPLIT:], d0_dram[k][:, D0SPLIT:])

            # ---- constants (host-provided) ----
            indsn = cpool.tile([128, NT], dt.float32, name="indsn")
            nc.sync.dma_start(indsn[:], c_indsn_dram[:])
            diag_f = cpool.tile([128, 16 * NT], dt.float32, name="diag_f")
            nc.sync.dma_start(diag_f[:], c_diagf_dram[:])
            iota16 = cpool.tile([128, 16], dt.uint16, name="iota16")
            nc.sync.dma_start(iota16[:], c_iota16_dram[:])
            diag256 = cpool.tile([128, 256], dt.float16, name="diag256")
            nc.sync.dma_start(diag256[:], c_diag256_dram[:])
            irev = cpool.tile([128, 16 * NT], dt.float16, name="irev")
            nc.sync.dma_start(irev[:], c_irev_dram[:])

            # prime the GPSIMD ucode library (indirect_copy) while input DMAs
            # run, so the IRAM load is off the critical path.
            prime = cpool.tile([128, 16], dt.float32, name="prime")
            zidx = cpool.tile([128, 1], dt.uint16, name="zidx")
            nc.vector.memset(zidx[:], 0)
            nc.gpsimd.indirect_copy(prime[:, :], diag_f[:, :16], zidx[:], True)

            # ---- per-direction accumulators ----
            t8_acc = [apool.tile([128, NSLOT * 8], dt.float16, name=f"t8_{d}") for d in range(2)]
            pi_acc = [apool.tile([128, NSLOT * 8], dt.uint16, name=f"pi_{d}") for d in range(2)]
            gt_acc = [apool.tile([128, 16 * NSLOT], dt.float16, name=f"gt_{d}") for d in range(2)]

            m_dir = [apool.tile([128, NT], dt.float32, name=f"mdir_{d}") for d in range(2)]
            scores0 = apool.tile([128, NT], dt.float32, name="scores0")

            # epilogue working tiles (slices written per chunk)
            v1g = [apool.tile([128, NT], dt.float32, name=f"v1g_{d}") for d in range(2)]
            v2g = [apool.tile([128, NT], dt.float32, name=f"v2g_{d}") for d in range(2)]
            pf = [apool.tile([128, NSLOT], dt.float32, name=f"pf_{d}") for d in range(2)]
            eqf = [apool.tile([128, 16 * NSLOT], dt.float16, name=f"eqf_{d}") for d in range(2)]
            wkf = [apool.tile([128, 16 * NSLOT], dt.float16, name=f"wkf_{d}") for d in range(2)]
            kred = [apool.tile([128, NSLOT], dt.float32, name=f"kred_{d}") for d in range(2)]
            kst = [apool.tile([128, NSLOT], dt.float32, name=f"kst_{d}") for d in range(2)]
            mst = [apool.tile([128, NSLOT], dt.float32, name=f"mst_{d}") for d in range(2)]
            acc1 = [apool.tile([128, NT], dt.float32, name=f"acc1_{d}") for d in range(2)]
            maskf = [apool.tile([128, NT], dt.uint8, name=f"maskf_{d}") for d in range(2)]

            m1_rep = apool.tile([128, M], dt.float32, name="m1_rep")

            def epilogue(dire, s0, s1):
                """Ratio test + match-index decode for tile slots [s0, s1)."""
                ns = s1 - s0
                t8a, pia = t8_acc[dire], pi_acc[dire]
                A3 = t8a[:, 8 * s0 : 8 * s1].rearrange("p (g e) -> p g e", e=8)
                P3 = pia[:, 8 * s0 : 8 * s1].rearrange("p (g e) -> p g e", e=8)

                sl = (slice(None), slice(s0, s1))
                nc.vector.tensor_copy(v1g[dire][sl], A3[:, :, 0])
                nc.vector.tensor_copy(v2g[dire][sl], A3[:, :, 1])
                nc.vector.tensor_copy(pf[dire][sl], P3[:, :, 0])

                # within-block position: first j with block[j] == v1
                gsl = (slice(None), slice(16 * s0, 16 * s1))
                v1b = (
                    t8a[:, 8 * s0 : 8 * s1]
                    .rearrange("p (g e) -> p g e", e=8)[:, :, 0:1]
                    .broadcast_to([128, ns, 16])
                )
                nc.vector.tensor_tensor(
                    eqf[dire][gsl].rearrange("p (g j) -> p g j", j=16),
                    gt_acc[dire][gsl].rearrange("p (g j) -> p g j", j=16),
                    v1b,
                    op=Alu.is_equal,
                )
                nc.vector.tensor_mul(wkf[dire][gsl], eqf[dire][gsl], irev[gsl])
                nc.vector.tensor_reduce(
                    kred[dire][sl],
                    wkf[dire][gsl].rearrange("p (g j) -> p g j", j=16),
                    axis=mybir.AxisListType.X,
                    op=Alu.max,
                )
                # k* = 16 - max_j((16-j) * eq_j)   (lowest matching j wins)
                nc.vector.tensor_scalar(
                    kst[dire][sl], kred[dire][sl], -1.0, 16.0,
                    op0=Alu.mult, op1=Alu.add,
                )
                # absolute column: m = 16*pf + k*
                nc.vector.scalar_tensor_tensor(
                    mst[dire][sl], pf[dire][sl], 16.0, kst[dire][sl],
                    op0=Alu.mult, op1=Alu.add,
                )

                # ratio test: dist1 <= r^2*dist2  <=>  v1 - r^2*v2 >= 1 - r^2
                nc.vector.scalar_tensor_tensor(
                    acc1[dire][sl], v2g[dire][sl], -RATIO2, v1g[dire][sl],
                    op0=Alu.mult, op1=Alu.add,
                )
                nc.vector.tensor_scalar(
                    maskf[dire][sl], acc1[dire][sl], 1.0 - RATIO2, None,
                    op0=Alu.is_ge,
                )
                if dire == 0:
                    sc = apool.tile([128, NT], dt.float32, name="sc")
                    nc.vector.tensor_scalar(
                        sc[sl], v1g[0][sl], 0.5, 0.5, op0=Alu.mult, op1=Alu.add
                    )
                    nc.vector.tensor_mul(scores0[sl], sc[sl], maskf[0][sl])
                nc.vector.memset(m_dir[dire][sl], -1.0)
                nc.vector.copy_predicated(m_dir[dire][sl], maskf[dire][sl], mst[dire][sl])

            for dire in range(2):
                lhs = d0_sb if dire == 0 else d1_sb
                rhs = d1_sb if dire == 0 else d0_sb
                t8a, pia = t8_acc[dire], pi_acc[dire]
                chunk = NSLOT // NCHUNK

                for t in range(NT):
                    s = t
                    X = xpool.tile([128, M], dt.float16, name=f"X_{dire}_{s}", tag="X")
                    for h in range(2):
                        P = ppool.tile([128, HALF], dt.float32, name=f"P_{dire}_{s}_{h}", tag="P")
                        for k in range(2):
                            for b in range(NBANK):
                                nc.tensor.matmul(
                                    P[:, 512 * b : 512 * (b + 1)],
                                    lhs[k][:, 128 * t : 128 * (t + 1)],
                                    rhs[k][:, HALF * h + 512 * b : HALF * h + 512 * (b + 1)],
                                    start=(k == 0),
                                    stop=(k == 1),
                                )
                        nc.scalar.copy(X[:, HALF * h : HALF * (h + 1)], P[:])

                    # block-local fold tree: F4[g] = max over X[16g : 16g+16)
                    Xv = X[:].rearrange("p (g h e) -> p g h e", h=2, e=8)
                    F1 = fpool.tile([128, M // 2], dt.float16, name=f"F1_{dire}_{s}", tag="F1")
                    nc.vector.tensor_max(
                        F1[:].rearrange("p (g e) -> p g e", e=8),
                        Xv[:, :, 0, :], Xv[:, :, 1, :],
                    )
                    F1v = F1[:].rearrange("p (g h e) -> p g h e", h=2, e=4)
                    F2 = fpool.tile([128, M // 4], dt.float16, name=f"F2_{dire}_{s}", tag="F2")
                    nc.vector.tensor_max(
                        F2[:].rearrange("p (g e) -> p g e", e=4),
                        F1v[:, :, 0, :], F1v[:, :, 1, :],
                    )
                    F2v = F2[:].rearrange("p (g h e) -> p g h e", h=2, e=2)
                    F3 = fpool.tile([128, M // 8], dt.float16, name=f"F3_{dire}_{s}", tag="F3")
                    nc.vector.tensor_max(
                        F3[:].rearrange("p (g e) -> p g e", e=2),
                        F2v[:, :, 0, :], F2v[:, :, 1, :],
                    )
                    F3v = F3[:].rearrange("p (g h) -> p g h", h=2)
                    F4 = fpool.tile([128, FW4], dt.float16, name=f"F4_{dire}_{s}", tag="F4")
                    nc.vector.tensor_max(F4[:], F3v[:, :, 0], F3v[:, :, 1])

                    t8_slot = t8a[:, 8 * s : 8 * s + 8]
                    pi_slot = pia[:, 8 * s : 8 * s + 8]
                    nc.vector.max(t8_slot, F4[:])
                    nc.vector.max_index(pi_slot, t8_slot, F4[:])

                    # gather the winner's 16-column block (per 16-partition
                    # group) and extract each row's own block via diag mask
                    bidx = fpool.tile([128, 16], dt.uint16, name=f"bi_{dire}_{s}", tag="bi")
                    nc.vector.scalar_tensor_tensor(
                        bidx[:], pi_slot[:, 0:1].broadcast_to([128, 16]), 16.0,
                        iota16[:], op0=Alu.mult, op1=Alu.add,
                    )
                    gblk = fpool.tile([128, 256], dt.float16, name=f"gb_{dire}_{s}", tag="gb")
                    nc.gpsimd.indirect_copy(gblk[:], X[:], bidx[:], True)
                    gmf = fpool.tile([128, 256], dt.float16, name=f"gm_{dire}_{s}", tag="gm")
                    nc.vector.tensor_mul(gmf[:], gblk[:], diag256[:])
                    with nc.allow_low_precision(
                        reason="diag-masked sum of one value + 15 zeros; exact"
                    ):
                        nc.vector.tensor_reduce(
                            gt_acc[dire][:, 16 * s : 16 * s + 16],
                            gmf[:].rearrange("p (u j) -> p u j", j=16),
                            axis=mybir.AxisListType.X,
                            op=Alu.add,
                        )

                    # dir-1: as soon as a chunk of tile slots is complete, run
                    # its epilogue, bounce its matches1 slice (r-major -> 32B
                    # runs) and broadcast it back while later tiles compute.
                    if dire == 1 and (t + 1) % chunk == 0:
                        c = (t + 1) // chunk - 1
                        s0, s1 = c * chunk, (c + 1) * chunk
                        epilogue(1, s0, s1)
                        m1_flat_ap = m1_bounce[:].rearrange("(r t) -> r t", t=NT)
                        nc.sync.dma_start(m1_flat_ap[:, s0:s1], m_dir[1][:, s0:s1])
                        nc.sync.dma_start(
                            m1_rep[:].rearrange("p (r t) -> p r t", t=NT)[:, :, s0:s1],
                            m1_flat_ap[:, s0:s1][None, :, :].partition_broadcast(128),
                        )

                if dire == 0:
                    epilogue(0, 0, NSLOT)
                    # scores are final after the dir-0 epilogue; ship them now
                    # (r-major DRAM layout; host transposes)
                    nc.sync.dma_start(
                        scores_dram[:].rearrange("(r t) -> r t", t=NT), scores0[:]
                    )
                    # gather indices for the mutual check, permuted into the
                    # r-major layout of m1_rep: m value v -> 32*(v%128) + v//128
                    safe = apool.tile([128, NT], dt.float32, name="safe")
                    nc.vector.tensor_scalar_max(safe[:], m_dir[0][:], 0.0)
                    safe16 = apool.tile([128, NT], dt.uint16, name="safe16")
                    nc.vector.tensor_copy(safe16[:], safe[:])
                    r16 = apool.tile([128, NT], dt.uint16, name="r16")
                    nc.vector.tensor_scalar(
                        r16[:], safe16[:], 127, 5,
                        op0=Alu.bitwise_and, op1=Alu.logical_shift_left,
                    )
                    t16 = apool.tile([128, NT], dt.uint16, name="t16")
                    nc.vector.tensor_scalar(
                        t16[:], safe16[:], 7, None, op0=Alu.logical_shift_right
                    )
                    idx16 = apool.tile([128, NT], dt.uint16, name="idx16")
                    nc.vector.tensor_tensor(idx16[:], r16[:], t16[:], op=Alu.bitwise_or)

            # ---- mutual check (m1_rep already broadcast per chunk) ----
            gm = apool.tile([128, 16 * NT], dt.float32, name="gm")
            nc.gpsimd.indirect_copy(gm[:], m1_rep[:], idx16[:], True)
            gmp = apool.tile([128, 16 * NT], dt.float32, name="gmp")
            nc.vector.tensor_mul(gmp[:], gm[:], diag_f[:])
            loop = apool.tile([128, NT], dt.float32, name="loop")
            nc.vector.tensor_reduce(
                loop[:],
                gmp[:].rearrange("p (j u) -> p j u", u=16),
                axis=mybir.AxisListType.X,
                op=Alu.add,
            )

            g1 = apool.tile([128, NT], dt.uint8, name="g1")
            nc.vector.tensor_scalar(g1[:], m_dir[0][:], -0.5, None, op0=Alu.is_gt)
            g2 = apool.tile([128, NT], dt.uint8, name="g2")
            nc.vector.tensor_tensor(g2[:], indsn[:], loop[:], op=Alu.is_equal)
            okm = apool.tile([128, NT], dt.uint8, name="okm")
            nc.vector.tensor_mul(okm[:], g1[:], g2[:])

            mfin = apool.tile([128, NT], dt.float32, name="mfin")
            nc.vector.memset(mfin[:], -1.0)
            nc.vector.copy_predicated(mfin[:], okm[:], m_dir[0][:])
            mi32 = apool.tile([128, NT], dt.int32, name="mi32")
            nc.vector.tensor_copy(mi32[:], mfin[:])

            nc.sync.dma_start(matches_dram[:].rearrange("(r t) -> r t", t=NT), mi32[:])
            if debug:
                nc.sync.dma_start(dbg_m0[:].rearrange("(r t) -> r t", t=NT), m_dir[0][:])
                nc.sync.dma_start(dbg_m1[:].rearrange("(r t) -> r t", t=NT), m_dir[1][:])
                nc.sync.dma_start(dbg_loop[:].rearrange("(r t) -> r t", t=NT), loop[:])
                nc.sync.dma_start(dbg_inds[:].rearrange("(r t) -> r t", t=NT), indsn[:])

    nc.compile()
    return nc


def _get_program():
    if "nc" not in _CACHE:
        _CACHE["nc"] = _build_program()
    return _CACHE["nc"]


def _make_consts():
    if "consts" in _CACHE:
        return _CACHE["consts"]
    p = np.arange(128)
    c_indsn = (128 * np.arange(NT)[None, :] + p[:, None]).astype(np.float32)
    diag = (np.arange(16)[None, :] == (p % 16)[:, None])  # [128, 16]
    c_diagf = np.tile(diag, (1, NT)).astype(np.float32)
    c_iota16 = np.tile(np.arange(16, dtype=np.uint16)[None, :], (128, 1))
    c_diag256 = (
        (np.arange(256)[None, :] % 16 == (p % 16)[:, None])
    ).astype(np.float16)
    c_irev = np.tile((16.0 - np.arange(16))[None, :], (128, NT)).astype(np.float16)
    consts = {
        "c_indsn": c_indsn, "c_diagf": c_diagf, "c_iota16": c_iota16,
        "c_diag256": c_diag256, "c_irev": c_irev,
    }
    _CACHE["consts"] = consts
    return consts


def _make_in_maps(descriptors0, descriptors1):
    consts = _make_consts()
    in_maps = []
    for c in range(B):
        a = np.ascontiguousarray(descriptors0[c].reshape(2, 128, N)).astype(
            ml_dtypes.bfloat16
        )
        bb = np.ascontiguousarray(descriptors1[c].reshape(2, 128, M)).astype(
            ml_dtypes.bfloat16
        )
        in_maps.append({"d0": a, "d1": bb, **consts})
    return in_maps


def kernel(descriptors0: np.ndarray, descriptors1: np.ndarray):
    from concourse.bass_utils import run_bass_kernel_spmd

    nc = _get_program()
    in_maps = _make_in_maps(descriptors0, descriptors1)
    res = run_bass_kernel_spmd(nc, in_maps, core_ids=list(range(B)))
    # outputs are written r-major ([128 partitions, NT tiles]); m = 128*t + r
    matches = np.stack([
        np.asarray(res.results[c]["matches"]).reshape(128, NT).T.reshape(-1)
        for c in range(B)
    ])
    scores = np.stack([
        np.asarray(res.results[c]["scores"]).reshape(128, NT).T.reshape(-1)
        for c in range(B)
    ])
    return matches.astype(np.int32), scores.astype(np.float32)


# revision 20
# speedup vs baseline: 1.0211x; 1.0211x over previous
"""Trainium2 Bass kernel for mutual-nearest-neighbor matching (Lowe ratio test).

Per-core layout: batch b=8 is sharded 1 batch element per NeuronCore (8 cores).
Each core computes, for its batch element:
  sim = d0^T @ d1          [n=4096, m=4096]   (bf16 matmuls, fp32 PSUM accum)
  top-2 + argmax along m  -> matches0 candidates + ratio mask + scores
  sim^T = d1^T @ d0        (second matmul direction)
  top-2 + argmax along n  -> matches1 candidates + ratio mask
  mutual check (fully local, via one small wrapped gather)
Outputs: matches int32 [4096], scores f32 [4096] per core; host stacks to [8, 4096].

v3 pipeline per 128-row tile:
  ACT evicts each PSUM half fp32 -> fp16 SBUF (exact to fp16, no bit tricks).
  DVE folds X [128, 4096] -> F4 [128, 256] with 4 levels of *within-16-block*
  pairwise max (block-local fan-in via 3D access patterns keeps step-1 inner
  runs, so levels 1-3 run in the packed 2x DVE mode).  F4[g] = max of block
  X[16g:16g+16).  Max8 gives the top-8 (so v1 and the cross-block runner-up
  v2), MaxIndex gives the winning block pf.  A per-tile gpsimd indirect_copy
  gathers the winner's 16-column block; a diag mask + reduce extracts each
  row's own block, and the epilogue recovers the exact column as
  16*pf + (first position equal to v1).
  No mantissa truncation anywhere: values are exact fp16, v2 is exact unless
  the top-2 share one 16-column block (then the ratio test sees the best
  value outside the block -- indistinguishable for any realistic margin).
Direction-1 epilogues run in chunks; each chunk's matches1 slice bounces to
DRAM (r-major, contiguous) and is broadcast to all partitions by a stride-0
DMA read while later tiles still compute.  The mutual check then needs one
small wrapped gather (indices permuted into the r-major layout).
"""

import sys

if "/opt/trn_rl_repo" not in sys.path:
    sys.path.insert(0, "/opt/trn_rl_repo")

import numpy as np
import ml_dtypes

B, D, N, M = 8, 256, 4096, 4096
NT = N // 128            # 32 row tiles per direction
HALF = M // 2            # 2048 columns per PSUM half-tile
NBANK = HALF // 512      # 4 matmul banks per half-tile
NSLOT = NT               # 32 row-tile slots per direction
RATIO2 = 0.8 * 0.8       # Lowe ratio threshold squared

FW4 = M // 16            # 256: width of the final fold array F4
NCHUNK = 4               # dir-1 epilogue chunks

_CACHE: dict = {}


def _build_program(debug=False):
    import concourse.mybir as mybir
    import concourse.tile as tile
    from concourse import bacc

    dt = mybir.dt
    Alu = mybir.AluOpType

    nc = bacc.Bacc("TRN2", target_bir_lowering=False, debug=False)

    d0_dram = nc.dram_tensor("d0", [2, 128, N], dt.bfloat16, kind="ExternalInput")
    d1_dram = nc.dram_tensor("d1", [2, 128, M], dt.bfloat16, kind="ExternalInput")
    matches_dram = nc.dram_tensor("matches", [N], dt.int32, kind="ExternalOutput")
    scores_dram = nc.dram_tensor("scores", [N], dt.float32, kind="ExternalOutput")
    m1_bounce = nc.dram_tensor("m1_bounce", [M], dt.float32)  # internal
    c_indsn_dram = nc.dram_tensor("c_indsn", [128, NT], dt.float32, kind="ExternalInput")
    c_diagf_dram = nc.dram_tensor("c_diagf", [128, 16 * NT], dt.float32, kind="ExternalInput")
    c_iota16_dram = nc.dram_tensor("c_iota16", [128, 16], dt.uint16, kind="ExternalInput")
    c_diag256_dram = nc.dram_tensor("c_diag256", [128, 256], dt.float16, kind="ExternalInput")
    c_irev_dram = nc.dram_tensor("c_irev", [128, 16 * NT], dt.float16, kind="ExternalInput")
    if debug:
        dbg_m0 = nc.dram_tensor("dbg_m0", [N], dt.float32, kind="ExternalOutput")
        dbg_m1 = nc.dram_tensor("dbg_m1", [M], dt.float32, kind="ExternalOutput")
        dbg_loop = nc.dram_tensor("dbg_loop", [N], dt.float32, kind="ExternalOutput")
        dbg_inds = nc.dram_tensor("dbg_inds", [N], dt.float32, kind="ExternalOutput")

    with tile.TileContext(nc) as tc:
        with (
            tc.tile_pool(name="w", bufs=1) as wpool,
            tc.tile_pool(name="consts", bufs=1) as cpool,
            tc.tile_pool(name="acc", bufs=1) as apool,
            tc.tile_pool(name="x", bufs=6) as xpool,
            tc.tile_pool(name="f", bufs=4) as fpool,
            tc.tile_pool(name="psum", bufs=2, space="PSUM") as ppool,
        ):
            # ---- load descriptors (already bf16, k-major [2, 128, N]) ----
            # d1 first (tile-0 matmuls need all of d1); d0 split so its first
            # columns arrive early.
            d0_sb = [wpool.tile([128, N], dt.bfloat16, name=f"d0_{k}") for k in range(2)]
            d1_sb = [wpool.tile([128, M], dt.bfloat16, name=f"d1_{k}") for k in range(2)]
            for k in range(2):
                nc.sync.dma_start(d1_sb[k][:], d1_dram[k])
            D0SPLIT = 512
            for k in range(2):
                nc.sync.dma_start(d0_sb[k][:, :D0SPLIT], d0_dram[k][:, :D0SPLIT])
            for k in range(2):
                nc.sync.dma_start(d0_sb[k][:, D0SPLIT:], d0_dram[k][:, D0SPLIT:])

            # ---- constants (host-provided) ----
            indsn = cpool.tile([128, NT], dt.float32, name="indsn")
            nc.sync.dma_start(indsn[:], c_indsn_dram[:])
            diag_f = cpool.tile([128, 16 * NT], dt.float32, name="diag_f")
            nc.sync.dma_start(diag_f[:], c_diagf_dram[:])
            iota16 = cpool.tile([128, 16], dt.uint16, name="iota16")
            nc.sync.dma_start(iota16[:], c_iota16_dram[:])
            diag256 = cpool.tile([128, 256], dt.float16, name="diag256")
            nc.sync.dma_start(diag256[:], c_diag256_dram[:])
            irev = cpool.tile([128, 16 * NT], dt.float16, name="irev")
            nc.sync.dma_start(irev[:], c_irev_dram[:])

            # prime the GPSIMD ucode library (indirect_copy) while input DMAs
            # run, so the IRAM load is off the critical path.
            prime = cpool.tile([128, 16], dt.float32, name="prime")
            zidx = cpool.tile([128, 1], dt.uint16, name="zidx")
            nc.vector.memset(zidx[:], 0)
            nc.gpsimd.indirect_copy(prime[:, :], diag_f[:, :16], zidx[:], True)

            # ---- per-direction accumulators ----
            t8_acc = [apool.tile([128, NSLOT * 8], dt.float16, name=f"t8_{d}") for d in range(2)]
            pi_acc = [apool.tile([128, NSLOT * 8], dt.uint16, name=f"pi_{d}") for d in range(2)]
            gt_acc = [apool.tile([128, 16 * NSLOT], dt.float16, name=f"gt_{d}") for d in range(2)]

            m_dir = [apool.tile([128, NT], dt.float32, name=f"mdir_{d}") for d in range(2)]
            scores0 = apool.tile([128, NT], dt.float32, name="scores0")

            # epilogue working tiles (slices written per chunk)
            v1g = [apool.tile([128, NT], dt.float32, name=f"v1g_{d}") for d in range(2)]
            v2g = [apool.tile([128, NT], dt.float32, name=f"v2g_{d}") for d in range(2)]
            pf = [apool.tile([128, NSLOT], dt.float32, name=f"pf_{d}") for d in range(2)]
            eqf = [apool.tile([128, 16 * NSLOT], dt.float16, name=f"eqf_{d}") for d in range(2)]
            wkf = [apool.tile([128, 16 * NSLOT], dt.float16, name=f"wkf_{d}") for d in range(2)]
            kred = [apool.tile([128, NSLOT], dt.float32, name=f"kred_{d}") for d in range(2)]
            kst = [apool.tile([128, NSLOT], dt.float32, name=f"kst_{d}") for d in range(2)]
            mst = [apool.tile([128, NSLOT], dt.float32, name=f"mst_{d}") for d in range(2)]
            acc1 = [apool.tile([128, NT], dt.float32, name=f"acc1_{d}") for d in range(2)]
            maskf = [apool.tile([128, NT], dt.uint8, name=f"maskf_{d}") for d in range(2)]

            m1_rep = apool.tile([128, M], dt.float32, name="m1_rep")

            def epilogue(dire, s0, s1):
                """Ratio test + match-index decode for tile slots [s0, s1)."""
                ns = s1 - s0
                t8a, pia = t8_acc[dire], pi_acc[dire]
                A3 = t8a[:, 8 * s0 : 8 * s1].rearrange("p (g e) -> p g e", e=8)
                P3 = pia[:, 8 * s0 : 8 * s1].rearrange("p (g e) -> p g e", e=8)

                sl = (slice(None), slice(s0, s1))
                nc.vector.tensor_copy(v1g[dire][sl], A3[:, :, 0])
                nc.vector.tensor_copy(v2g[dire][sl], A3[:, :, 1])
                nc.vector.tensor_copy(pf[dire][sl], P3[:, :, 0])

                # within-block position: first j with block[j] == v1
                gsl = (slice(None), slice(16 * s0, 16 * s1))
                v1b = (
                    t8a[:, 8 * s0 : 8 * s1]
                    .rearrange("p (g e) -> p g e", e=8)[:, :, 0:1]
                    .broadcast_to([128, ns, 16])
                )
                nc.vector.tensor_tensor(
                    eqf[dire][gsl].rearrange("p (g j) -> p g j", j=16),
                    gt_acc[dire][gsl].rearrange("p (g j) -> p g j", j=16),
                    v1b,
                    op=Alu.is_equal,
                )
                nc.vector.tensor_mul(wkf[dire][gsl], eqf[dire][gsl], irev[gsl])
                nc.vector.tensor_reduce(
                    kred[dire][sl],
                    wkf[dire][gsl].rearrange("p (g j) -> p g j", j=16),
                    axis=mybir.AxisListType.X,
                    op=Alu.max,
                )
                # k* = 16 - max_j((16-j) * eq_j)   (lowest matching j wins)
                nc.vector.tensor_scalar(
                    kst[dire][sl], kred[dire][sl], -1.0, 16.0,
                    op0=Alu.mult, op1=Alu.add,
                )
                # absolute column: m = 16*pf + k*
                nc.vector.scalar_tensor_tensor(
                    mst[dire][sl], pf[dire][sl], 16.0, kst[dire][sl],
                    op0=Alu.mult, op1=Alu.add,
                )

                # ratio test: dist1 <= r^2*dist2  <=>  v1 - r^2*v2 >= 1 - r^2
                nc.vector.scalar_tensor_tensor(
                    acc1[dire][sl], v2g[dire][sl], -RATIO2, v1g[dire][sl],
                    op0=Alu.mult, op1=Alu.add,
                )
                nc.vector.tensor_scalar(
                    maskf[dire][sl], acc1[dire][sl], 1.0 - RATIO2, None,
                    op0=Alu.is_ge,
                )
                if dire == 0:
                    sc = apool.tile([128, NT], dt.float32, name="sc")
                    nc.vector.tensor_scalar(
                        sc[sl], v1g[0][sl], 0.5, 0.5, op0=Alu.mult, op1=Alu.add
                    )
                    nc.vector.tensor_mul(scores0[sl], sc[sl], maskf[0][sl])
                nc.vector.memset(m_dir[dire][sl], -1.0)
                nc.vector.copy_predicated(m_dir[dire][sl], maskf[dire][sl], mst[dire][sl])

            for dire in range(2):
                lhs = d0_sb if dire == 0 else d1_sb
                rhs = d1_sb if dire == 0 else d0_sb
                t8a, pia = t8_acc[dire], pi_acc[dire]
                chunk = NSLOT // NCHUNK
                pending_extract = []

                for t in range(NT):
                    s = t
                    X = xpool.tile([128, M], dt.float16, name=f"X_{dire}_{s}", tag="X")
                    for h in range(2):
                        P = ppool.tile([128, HALF], dt.float32, name=f"P_{dire}_{s}_{h}", tag="P")
                        for k in range(2):
                            for b in range(NBANK):
                                nc.tensor.matmul(
                                    P[:, 512 * b : 512 * (b + 1)],
                                    lhs[k][:, 128 * t : 128 * (t + 1)],
                                    rhs[k][:, HALF * h + 512 * b : HALF * h + 512 * (b + 1)],
                                    start=(k == 0),
                                    stop=(k == 1),
                                )
                        nc.scalar.copy(X[:, HALF * h : HALF * (h + 1)], P[:])

                    # block-local fold tree: F4[g] = max over X[16g : 16g+16)
                    Xv = X[:].rearrange("p (g h e) -> p g h e", h=2, e=8)
                    F1 = fpool.tile([128, M // 2], dt.float16, name=f"F1_{dire}_{s}", tag="F1")
                    nc.vector.tensor_max(
                        F1[:].rearrange("p (g e) -> p g e", e=8),
                        Xv[:, :, 0, :], Xv[:, :, 1, :],
                    )
                    F1v = F1[:].rearrange("p (g h e) -> p g h e", h=2, e=4)
                    F2 = fpool.tile([128, M // 4], dt.float16, name=f"F2_{dire}_{s}", tag="F2")
                    nc.vector.tensor_max(
                        F2[:].rearrange("p (g e) -> p g e", e=4),
                        F1v[:, :, 0, :], F1v[:, :, 1, :],
                    )
                    F2v = F2[:].rearrange("p (g h e) -> p g h e", h=2, e=2)
                    F3 = fpool.tile([128, M // 8], dt.float16, name=f"F3_{dire}_{s}", tag="F3")
                    nc.vector.tensor_max(
                        F3[:].rearrange("p (g e) -> p g e", e=2),
                        F2v[:, :, 0, :], F2v[:, :, 1, :],
                    )
                    F3v = F3[:].rearrange("p (g h) -> p g h", h=2)
                    F4 = fpool.tile([128, FW4], dt.float16, name=f"F4_{dire}_{s}", tag="F4")
                    nc.vector.tensor_max(F4[:], F3v[:, :, 0], F3v[:, :, 1])

                    t8_slot = t8a[:, 8 * s : 8 * s + 8]
                    pi_slot = pia[:, 8 * s : 8 * s + 8]
                    nc.vector.max(t8_slot, F4[:])
                    nc.vector.max_index(pi_slot, t8_slot, F4[:])

                    # gather the winner's 16-column block (per 16-partition
                    # group) and extract each row's own block via diag mask
                    bidx = fpool.tile([128, 16], dt.uint16, name=f"bi_{dire}_{s}", tag="bi")
                    nc.vector.scalar_tensor_tensor(
                        bidx[:], pi_slot[:, 0:1].broadcast_to([128, 16]), 16.0,
                        iota16[:], op0=Alu.mult, op1=Alu.add,
                    )
                    gblk = fpool.tile([128, 256], dt.float16, name=f"gb_{dire}_{s}", tag="gb")
                    nc.gpsimd.indirect_copy(gblk[:], X[:], bidx[:], True)

                    # defer the gather extraction by one tile: the DVE queue
                    # is strict FIFO, so queueing the mul right away would
                    # head-of-line-block the next tile's folds on the gather
                    # semaphore.  Emitting it one tile later hides the gather.
                    def extract(s=s, gblk=gblk, dire=dire):
                        gmf = fpool.tile([128, 256], dt.float16, name=f"gm_{dire}_{s}", tag="gm")
                        nc.vector.tensor_mul(gmf[:], gblk[:], diag256[:])
                        with nc.allow_low_precision(
                            reason="diag-masked sum of one value + 15 zeros; exact"
                        ):
                            nc.vector.tensor_reduce(
                                gt_acc[dire][:, 16 * s : 16 * s + 16],
                                gmf[:].rearrange("p (u j) -> p u j", j=16),
                                axis=mybir.AxisListType.X,
                                op=Alu.add,
                            )
                    pending_extract.append(extract)
                    if len(pending_extract) > 1:
                        pending_extract.pop(0)()
                    if t == NT - 1:
                        pending_extract.pop(0)()

                    # dir-1: as soon as a chunk of tile slots is complete, run
                    # its epilogue, bounce its matches1 slice (r-major -> 32B
                    # runs) and broadcast it back while later tiles compute.
                    if dire == 1 and (t + 1) % chunk == 0:
                        while pending_extract:
                            pending_extract.pop(0)()
                        c = (t + 1) // chunk - 1
                        s0, s1 = c * chunk, (c + 1) * chunk
                        epilogue(1, s0, s1)
                        m1_flat_ap = m1_bounce[:].rearrange("(r t) -> r t", t=NT)
                        nc.sync.dma_start(m1_flat_ap[:, s0:s1], m_dir[1][:, s0:s1])
                        nc.sync.dma_start(
                            m1_rep[:].rearrange("p (r t) -> p r t", t=NT)[:, :, s0:s1],
                            m1_flat_ap[:, s0:s1][None, :, :].partition_broadcast(128),
                        )

                if dire == 0:
                    epilogue(0, 0, NSLOT)
                    # scores are final after the dir-0 epilogue; ship them now
                    # (r-major DRAM layout; host transposes)
                    nc.sync.dma_start(
                        scores_dram[:].rearrange("(r t) -> r t", t=NT), scores0[:]
                    )
                    # gather indices for the mutual check, permuted into the
                    # r-major layout of m1_rep: m value v -> 32*(v%128) + v//128
                    safe = apool.tile([128, NT], dt.float32, name="safe")
                    nc.vector.tensor_scalar_max(safe[:], m_dir[0][:], 0.0)
                    safe16 = apool.tile([128, NT], dt.uint16, name="safe16")
                    nc.vector.tensor_copy(safe16[:], safe[:])
                    r16 = apool.tile([128, NT], dt.uint16, name="r16")
                    nc.vector.tensor_scalar(
                        r16[:], safe16[:], 127, 5,
                        op0=Alu.bitwise_and, op1=Alu.logical_shift_left,
                    )
                    t16 = apool.tile([128, NT], dt.uint16, name="t16")
                    nc.vector.tensor_scalar(
                        t16[:], safe16[:], 7, None, op0=Alu.logical_shift_right
                    )
                    idx16 = apool.tile([128, NT], dt.uint16, name="idx16")
                    nc.vector.tensor_tensor(idx16[:], r16[:], t16[:], op=Alu.bitwise_or)

            # ---- mutual check (m1_rep already broadcast per chunk) ----
            gm = apool.tile([128, 16 * NT], dt.float32, name="gm")
            nc.gpsimd.indirect_copy(gm[:], m1_rep[:], idx16[:], True)
            gmp = apool.tile([128, 16 * NT], dt.float32, name="gmp")
            nc.vector.tensor_mul(gmp[:], gm[:], diag_f[:])
            loop = apool.tile([128, NT], dt.float32, name="loop")
            nc.vector.tensor_reduce(
                loop[:],
                gmp[:].rearrange("p (j u) -> p j u", u=16),
                axis=mybir.AxisListType.X,
                op=Alu.add,
            )

            g1 = apool.tile([128, NT], dt.uint8, name="g1")
            nc.vector.tensor_scalar(g1[:], m_dir[0][:], -0.5, None, op0=Alu.is_gt)
            g2 = apool.tile([128, NT], dt.uint8, name="g2")
            nc.vector.tensor_tensor(g2[:], indsn[:], loop[:], op=Alu.is_equal)
            okm = apool.tile([128, NT], dt.uint8, name="okm")
            nc.vector.tensor_mul(okm[:], g1[:], g2[:])

            mfin = apool.tile([128, NT], dt.float32, name="mfin")
            nc.vector.memset(mfin[:], -1.0)
            nc.vector.copy_predicated(mfin[:], okm[:], m_dir[0][:])
            mi32 = apool.tile([128, NT], dt.int32, name="mi32")
            nc.vector.tensor_copy(mi32[:], mfin[:])

            nc.sync.dma_start(matches_dram[:].rearrange("(r t) -> r t", t=NT), mi32[:])
            if debug:
                nc.sync.dma_start(dbg_m0[:].rearrange("(r t) -> r t", t=NT), m_dir[0][:])
                nc.sync.dma_start(dbg_m1[:].rearrange("(r t) -> r t", t=NT), m_dir[1][:])
                nc.sync.dma_start(dbg_loop[:].rearrange("(r t) -> r t", t=NT), loop[:])
                nc.sync.dma_start(dbg_inds[:].rearrange("(r t) -> r t", t=NT), indsn[:])

    nc.compile()
    return nc


def _get_program():
    if "nc" not in _CACHE:
        _CACHE["nc"] = _build_program()
    return _CACHE["nc"]


def _make_consts():
    if "consts" in _CACHE:
        return _CACHE["consts"]
    p = np.arange(128)
    c_indsn = (128 * np.arange(NT)[None, :] + p[:, None]).astype(np.float32)
    diag = (np.arange(16)[None, :] == (p % 16)[:, None])  # [128, 16]
    c_diagf = np.tile(diag, (1, NT)).astype(np.float32)
    c_iota16 = np.tile(np.arange(16, dtype=np.uint16)[None, :], (128, 1))
    c_diag256 = (
        (np.arange(256)[None, :] % 16 == (p % 16)[:, None])
    ).astype(np.float16)
    c_irev = np.tile((16.0 - np.arange(16))[None, :], (128, NT)).astype(np.float16)
    consts = {
        "c_indsn": c_indsn, "c_diagf": c_diagf, "c_iota16": c_iota16,
        "c_diag256": c_diag256, "c_irev": c_irev,
    }
    _CACHE["consts"] = consts
    return consts


def _make_in_maps(descriptors0, descriptors1):
    consts = _make_consts()
    in_maps = []
    for c in range(B):
        a = np.ascontiguousarray(descriptors0[c].reshape(2, 128, N)).astype(
            ml_dtypes.bfloat16
        )
        bb = np.ascontiguousarray(descriptors1[c].reshape(2, 128, M)).astype(
            ml_dtypes.bfloat16
        )
        in_maps.append({"d0": a, "d1": bb, **consts})
    return in_maps


def kernel(descriptors0: np.ndarray, descriptors1: np.ndarray):
    from concourse.bass_utils import run_bass_kernel_spmd

    nc = _get_program()
    in_maps = _make_in_maps(descriptors0, descriptors1)
    res = run_bass_kernel_spmd(nc, in_maps, core_ids=list(range(B)))
    # outputs are written r-major ([128 partitions, NT tiles]); m = 128*t + r
    matches = np.stack([
        np.asarray(res.results[c]["matches"]).reshape(128, NT).T.reshape(-1)
        for c in range(B)
    ])
    scores = np.stack([
        np.asarray(res.results[c]["scores"]).reshape(128, NT).T.reshape(-1)
        for c in range(B)
    ])
    return matches.astype(np.int32), scores.astype(np.float32)


# revision 24
# speedup vs baseline: 1.6635x; 1.6292x over previous
"""Trainium2 Bass kernel for mutual-nearest-neighbor matching (Lowe ratio test).

Per-core layout: batch b=8 is sharded 1 batch element per NeuronCore (8 cores).
Each core computes, for its batch element:
  sim = d0^T @ d1          [n=4096, m=4096]   (bf16 matmuls, fp32 PSUM accum)
  top-2 + argmax along m  -> matches0 candidates + ratio mask + scores
  sim^T = d1^T @ d0        (second matmul direction)
  top-2 + argmax along n  -> matches1 candidates + ratio mask
  mutual check (fully local, via one small wrapped gather)
Outputs: matches int32 [4096], scores f32 [4096] per core; host stacks to [8, 4096].

v2 changes vs baseline:
  - PSUM halves are evicted by ACT to fp16 (10-bit mantissa) instead of bf16;
    4 low mantissa bits are reserved for fold-branch metadata (comb-16), which
    still leaves 6 value bits -- better precision than the bf16/3-bit baseline.
  - 4 fold levels (4096 -> 256) before Max8/MaxIndex, shrinking the 1x-rate
    top-8 scan from 512 to 256 columns.
  - The two full-width mantissa-cleanup TensorScalar passes can run on the
    (otherwise idle) GPSIMD engine instead of DVE (AND_ON_GPSIMD flag).
  - Direction-1 epilogue is chunked; each chunk's matches1 slice is bounced to
    DRAM and partition-broadcast while later tiles still compute, removing the
    ~65us serialized mutual-check tail.
  - Input DMAs are split so the first row-tile's matmuls start earlier.
"""

import sys

if "/opt/trn_rl_repo" not in sys.path:
    sys.path.insert(0, "/opt/trn_rl_repo")

import numpy as np
import ml_dtypes

B, D, N, M = 8, 256, 4096, 4096
NT = N // 128            # 32 row tiles per direction
HALF = M // 2            # 2048 columns per PSUM half-tile
NBANK = HALF // 512      # 4 matmul banks per half-tile
NSLOT = NT               # 32 row-tile slots per direction
RATIO2 = 0.8 * 0.8       # Lowe ratio threshold squared

FW4 = M // 16            # 256: width of the final fold array F4
MASK = 0xFFF0            # keep 6 fp16 mantissa bits, reserve 4 for branch bits
NCHUNK = 4               # dir-1 epilogue chunks (NSLOT must divide evenly)
AND_ON_GPSIMD = False    # GPSIMD lacks TENSOR_SCALAR support on TRN2 codegen

_CACHE: dict = {}


def _build_program(debug=False):
    import concourse.mybir as mybir
    import concourse.tile as tile
    from concourse import bacc

    dt = mybir.dt
    Alu = mybir.AluOpType

    nc = bacc.Bacc("TRN2", target_bir_lowering=False, debug=False)

    d0_dram = nc.dram_tensor("d0", [2, 128, N], dt.bfloat16, kind="ExternalInput")
    d1_dram = nc.dram_tensor("d1", [2, 128, M], dt.bfloat16, kind="ExternalInput")
    matches_dram = nc.dram_tensor("matches", [N], dt.int32, kind="ExternalOutput")
    scores_dram = nc.dram_tensor("scores", [N], dt.float32, kind="ExternalOutput")
    m1_bounce = nc.dram_tensor("m1_bounce", [M], dt.float32)  # internal
    c_indsn_dram = nc.dram_tensor("c_indsn", [128, NT], dt.float32, kind="ExternalInput")
    c_diagf_dram = nc.dram_tensor("c_diagf", [128, 16 * NT], dt.float32, kind="ExternalInput")
    if debug:
        dbg_m0 = nc.dram_tensor("dbg_m0", [N], dt.float32, kind="ExternalOutput")
        dbg_m1 = nc.dram_tensor("dbg_m1", [M], dt.float32, kind="ExternalOutput")
        dbg_loop = nc.dram_tensor("dbg_loop", [N], dt.float32, kind="ExternalOutput")
        dbg_inds = nc.dram_tensor("dbg_inds", [N], dt.float32, kind="ExternalOutput")

    with tile.TileContext(nc) as tc:
        with (
            tc.tile_pool(name="w", bufs=1) as wpool,
            tc.tile_pool(name="consts", bufs=1) as cpool,
            tc.tile_pool(name="acc", bufs=1) as apool,
            tc.tile_pool(name="x", bufs=4) as xpool,
            tc.tile_pool(name="f", bufs=3) as fpool,
            tc.tile_pool(name="psum", bufs=2, space="PSUM") as ppool,
        ):
            # ---- load descriptors (already bf16, k-major [2, 128, N]) ----
            # d1 first (tile-0 matmuls need all of d1); d0 split so its first
            # columns arrive early.
            d0_sb = [wpool.tile([128, N], dt.bfloat16, name=f"d0_{k}") for k in range(2)]
            d1_sb = [wpool.tile([128, M], dt.bfloat16, name=f"d1_{k}") for k in range(2)]
            for k in range(2):
                nc.sync.dma_start(d1_sb[k][:], d1_dram[k])
            D0SPLIT = 512
            for k in range(2):
                nc.sync.dma_start(d0_sb[k][:, :D0SPLIT], d0_dram[k][:, :D0SPLIT])
            for k in range(2):
                nc.sync.dma_start(d0_sb[k][:, D0SPLIT:], d0_dram[k][:, D0SPLIT:])

            # ---- constants (host-provided) ----
            indsn = cpool.tile([128, NT], dt.float32, name="indsn")
            nc.sync.dma_start(indsn[:], c_indsn_dram[:])
            diag_f = cpool.tile([128, 16 * NT], dt.float32, name="diag_f")
            nc.sync.dma_start(diag_f[:], c_diagf_dram[:])

            # prime the GPSIMD ucode library (indirect_copy) while input DMAs
            # run, so the IRAM load is off the critical path at the end.
            prime = cpool.tile([128, 16], dt.float32, name="prime")
            zidx = cpool.tile([128, 1], dt.uint16, name="zidx")
            nc.vector.memset(zidx[:], 0)
            nc.gpsimd.indirect_copy(prime[:, :], diag_f[:, :16], zidx[:], True)

            # ---- per-direction accumulators ----
            t8_acc = [apool.tile([128, NSLOT * 8], dt.float16, name=f"t8_{d}") for d in range(2)]
            pi_acc = [apool.tile([128, NSLOT * 8], dt.uint16, name=f"pi_{d}") for d in range(2)]

            m_dir = [apool.tile([128, NT], dt.float32, name=f"mdir_{d}") for d in range(2)]
            scores0 = apool.tile([128, NT], dt.float32, name="scores0")

            # epilogue working tiles (slices written per chunk)
            t8c = [apool.tile([128, NSLOT * 8], dt.uint16, name=f"t8c_{d}") for d in range(2)]
            v1g = [apool.tile([128, NT], dt.float32, name=f"v1g_{d}") for d in range(2)]
            v2g = [apool.tile([128, NT], dt.float32, name=f"v2g_{d}") for d in range(2)]
            pf = [apool.tile([128, NSLOT], dt.float32, name=f"pf_{d}") for d in range(2)]
            bu = [[apool.tile([128, NSLOT], dt.uint16, name=f"b{i}u_{d}") for i in range(4)]
                  for d in range(2)]
            bf = [[apool.tile([128, NSLOT], dt.float32, name=f"b{i}f_{d}") for i in range(4)]
                  for d in range(2)]
            mst = [apool.tile([128, NSLOT], dt.float32, name=f"mst_{d}") for d in range(2)]
            acc1 = [apool.tile([128, NT], dt.float32, name=f"acc1_{d}") for d in range(2)]
            maskf = [apool.tile([128, NT], dt.uint8, name=f"maskf_{d}") for d in range(2)]

            m1_rep = apool.tile([128, M], dt.float32, name="m1_rep")

            def epilogue(dire, s0, s1):
                """Ratio test + match-index decode for tile slots [s0, s1)."""
                ns = s1 - s0
                t8a, pia = t8_acc[dire], pi_acc[dire]
                # strip the embedded index bits from the stored top-8 values
                nc.vector.tensor_scalar(
                    t8c[dire][:, 8 * s0 : 8 * s1].bitcast(dt.uint16),
                    t8a[:, 8 * s0 : 8 * s1].bitcast(dt.uint16), MASK, None,
                    op0=Alu.bitwise_and,
                )
                A3 = t8c[dire][:, 8 * s0 : 8 * s1].bitcast(dt.float16).rearrange(
                    "p (g e) -> p g e", e=8)
                A3u = t8a[:, 8 * s0 : 8 * s1].bitcast(dt.uint16).rearrange(
                    "p (g e) -> p g e", e=8)
                P3 = pia[:, 8 * s0 : 8 * s1].rearrange("p (g e) -> p g e", e=8)

                sl = (slice(None), slice(s0, s1))
                nc.vector.tensor_copy(v1g[dire][sl], A3[:, :, 0])
                nc.vector.tensor_copy(v2g[dire][sl], A3[:, :, 1])
                nc.vector.tensor_copy(pf[dire][sl], P3[:, :, 0])

                # decode the winner's branch bits: bit0 (X level, weight 2048),
                # bit1 (F1 level, raw 2 -> weight 1024), bit2 (F2 level, raw 4
                # -> weight 512), bit3 (F3 level, raw 8 -> weight 256)
                for i, rawbit in enumerate([1, 2, 4, 8]):
                    nc.vector.tensor_scalar(
                        bu[dire][i][sl], A3u[:, :, 0], rawbit, None,
                        op0=Alu.bitwise_and,
                    )
                    nc.vector.tensor_copy(bf[dire][i][sl], bu[dire][i][sl])

                # absolute column: m = p + 2048*b0 + 1024*(b1/2) + 512*(b2/4)
                #                        + 256*(b3/8)
                nc.vector.scalar_tensor_tensor(
                    mst[dire][sl], bf[dire][0][sl], 2048.0, pf[dire][sl],
                    op0=Alu.mult, op1=Alu.add,
                )
                for i, w in [(1, 512.0), (2, 128.0), (3, 32.0)]:
                    nc.vector.scalar_tensor_tensor(
                        mst[dire][sl], bf[dire][i][sl], w, mst[dire][sl],
                        op0=Alu.mult, op1=Alu.add,
                    )

                # ratio test: dist1 <= r^2*dist2  <=>  v1 - r^2*v2 >= 1 - r^2
                nc.vector.scalar_tensor_tensor(
                    acc1[dire][sl], v2g[dire][sl], -RATIO2, v1g[dire][sl],
                    op0=Alu.mult, op1=Alu.add,
                )
                nc.vector.tensor_scalar(
                    maskf[dire][sl], acc1[dire][sl], 1.0 - RATIO2, None,
                    op0=Alu.is_ge,
                )
                if dire == 0:
                    sc = apool.tile([128, NT], dt.float32, name="sc")
                    nc.vector.tensor_scalar(
                        sc[sl], v1g[0][sl], 0.5, 0.5, op0=Alu.mult, op1=Alu.add
                    )
                    nc.vector.tensor_mul(scores0[sl], sc[sl], maskf[0][sl])
                nc.vector.memset(m_dir[dire][sl], -1.0)
                nc.vector.copy_predicated(m_dir[dire][sl], maskf[dire][sl], mst[dire][sl])

            and_eng = nc.gpsimd if AND_ON_GPSIMD else nc.vector

            for dire in range(2):
                lhs = d0_sb if dire == 0 else d1_sb
                rhs = d1_sb if dire == 0 else d0_sb
                t8a, pia = t8_acc[dire], pi_acc[dire]
                chunk = NSLOT // NCHUNK

                for t in range(NT):
                    s = t
                    X = xpool.tile([128, M], dt.float16, name=f"X_{dire}_{s}", tag="X")
                    for h in range(2):
                        P = ppool.tile([128, HALF], dt.float32, name=f"P_{dire}_{s}_{h}", tag="P")
                        for k in range(2):
                            for b in range(NBANK):
                                nc.tensor.matmul(
                                    P[:, 512 * b : 512 * (b + 1)],
                                    lhs[k][:, 128 * t : 128 * (t + 1)],
                                    rhs[k][:, HALF * h + 512 * b : HALF * h + 512 * (b + 1)],
                                    start=(k == 0),
                                    stop=(k == 1),
                                )
                        nc.scalar.copy(X[:, HALF * h : HALF * (h + 1)], P[:])
                    # bit-packed folds: truncate the 4 low mantissa bits and OR a
                    # fold-branch bit into each fold's right operand.  The fold
                    # winner then carries its own comb-branch bits.
                    Xu = X[:].bitcast(dt.uint16)
                    XL = fpool.tile([128, M // 2], dt.float16, name=f"XL_{dire}_{s}", tag="XL")
                    and_eng.tensor_scalar(
                        XL[:].bitcast(dt.uint16), Xu[:, : M // 2], MASK, None,
                        op0=Alu.bitwise_and,
                    )
                    XR = fpool.tile([128, M // 2], dt.float16, name=f"XR_{dire}_{s}", tag="XR")
                    and_eng.tensor_scalar(
                        XR[:].bitcast(dt.uint16), Xu[:, M // 2 :], MASK, 1,
                        op0=Alu.bitwise_and, op1=Alu.bitwise_or,
                    )
                    F1 = fpool.tile([128, M // 2], dt.float16, name=f"F1_{dire}_{s}", tag="F1")
                    nc.vector.tensor_max(F1[:], XL[:], XR[:])
                    FR2 = fpool.tile([128, M // 4], dt.float16, name=f"FR2_{dire}_{s}", tag="FR2")
                    nc.vector.tensor_scalar(
                        FR2[:].bitcast(dt.uint16), F1[:].bitcast(dt.uint16)[:, M // 4 :], 2, None,
                        op0=Alu.bitwise_or,
                    )
                    F2 = fpool.tile([128, M // 4], dt.float16, name=f"F2_{dire}_{s}", tag="F2")
                    nc.vector.tensor_max(F2[:], F1[:, : M // 4], FR2[:])
                    FR3 = fpool.tile([128, M // 8], dt.float16, name=f"FR3_{dire}_{s}", tag="FR3")
                    nc.vector.tensor_scalar(
                        FR3[:].bitcast(dt.uint16), F2[:].bitcast(dt.uint16)[:, M // 8 :], 4, None,
                        op0=Alu.bitwise_or,
                    )
                    F3 = fpool.tile([128, M // 8], dt.float16, name=f"F3_{dire}_{s}", tag="F3")
                    nc.vector.tensor_max(F3[:], F2[:, : M // 8], FR3[:])
                    FR4 = fpool.tile([128, FW4], dt.float16, name=f"FR4_{dire}_{s}", tag="FR4")
                    nc.vector.tensor_scalar(
                        FR4[:].bitcast(dt.uint16), F3[:].bitcast(dt.uint16)[:, FW4:], 8, None,
                        op0=Alu.bitwise_or,
                    )
                    F4 = fpool.tile([128, FW4], dt.float16, name=f"F4_{dire}_{s}", tag="F4")
                    nc.vector.tensor_max(F4[:], F3[:, :FW4], FR4[:])

                    t8_slot = t8a[:, 8 * s : 8 * s + 8]
                    pi_slot = pia[:, 8 * s : 8 * s + 8]
                    nc.vector.max(t8_slot, F4[:])
                    nc.vector.max_index(pi_slot, t8_slot, F4[:])

                    # dir-1: as soon as a chunk of tile slots is complete, run
                    # its epilogue and ship its matches1 slice (r-major DRAM
                    # layout -> 32B-contiguous runs instead of 4B scatter) so
                    # only the final broadcast remains at the end.
                    if dire == 1 and (t + 1) % chunk == 0:
                        c = (t + 1) // chunk - 1
                        s0, s1 = c * chunk, (c + 1) * chunk
                        epilogue(1, s0, s1)
                        m1_flat_ap = m1_bounce[:].rearrange("(r t) -> r t", t=NT)
                        nc.sync.dma_start(m1_flat_ap[:, s0:s1], m_dir[1][:, s0:s1])

                if dire == 0:
                    epilogue(0, 0, NSLOT)
                    # scores are final after the dir-0 epilogue; ship them now
                    # (r-major DRAM layout; host transposes)
                    nc.sync.dma_start(
                        scores_dram[:].rearrange("(r t) -> r t", t=NT), scores0[:]
                    )
                    # gather indices for the mutual check, permuted into the
                    # r-major layout of m1_rep: m value v -> 32*(v%128) + v//128
                    safe = apool.tile([128, NT], dt.float32, name="safe")
                    nc.vector.tensor_scalar_max(safe[:], m_dir[0][:], 0.0)
                    safe16 = apool.tile([128, NT], dt.uint16, name="safe16")
                    nc.vector.tensor_copy(safe16[:], safe[:])
                    r16 = apool.tile([128, NT], dt.uint16, name="r16")
                    nc.vector.tensor_scalar(
                        r16[:], safe16[:], 127, 5,
                        op0=Alu.bitwise_and, op1=Alu.logical_shift_left,
                    )
                    t16 = apool.tile([128, NT], dt.uint16, name="t16")
                    nc.vector.tensor_scalar(
                        t16[:], safe16[:], 7, None, op0=Alu.logical_shift_right
                    )
                    idx16 = apool.tile([128, NT], dt.uint16, name="idx16")
                    nc.vector.tensor_tensor(idx16[:], r16[:], t16[:], op=Alu.bitwise_or)

            # ---- mutual check ----
            # broadcast the (r-major) bounce to all partitions in one DMA
            # (stride-0 partition read) -- much faster than the Q7 daisy chain
            nc.sync.dma_start(
                m1_rep[:], m1_bounce[:][None, :].partition_broadcast(128)
            )
            gm = apool.tile([128, 16 * NT], dt.float32, name="gm")
            nc.gpsimd.indirect_copy(gm[:], m1_rep[:], idx16[:], True)
            gmp = apool.tile([128, 16 * NT], dt.float32, name="gmp")
            nc.vector.tensor_mul(gmp[:], gm[:], diag_f[:])
            loop = apool.tile([128, NT], dt.float32, name="loop")
            nc.vector.tensor_reduce(
                loop[:],
                gmp[:].rearrange("p (j u) -> p j u", u=16),
                axis=mybir.AxisListType.X,
                op=Alu.add,
            )

            g1 = apool.tile([128, NT], dt.uint8, name="g1")
            nc.vector.tensor_scalar(g1[:], m_dir[0][:], -0.5, None, op0=Alu.is_gt)
            g2 = apool.tile([128, NT], dt.uint8, name="g2")
            nc.vector.tensor_tensor(g2[:], indsn[:], loop[:], op=Alu.is_equal)
            okm = apool.tile([128, NT], dt.uint8, name="okm")
            nc.vector.tensor_mul(okm[:], g1[:], g2[:])

            mfin = apool.tile([128, NT], dt.float32, name="mfin")
            nc.vector.memset(mfin[:], -1.0)
            nc.vector.copy_predicated(mfin[:], okm[:], m_dir[0][:])
            mi32 = apool.tile([128, NT], dt.int32, name="mi32")
            nc.vector.tensor_copy(mi32[:], mfin[:])

            nc.sync.dma_start(matches_dram[:].rearrange("(r t) -> r t", t=NT), mi32[:])
            if debug:
                nc.sync.dma_start(dbg_m0[:].rearrange("(r t) -> r t", t=NT), m_dir[0][:])
                nc.sync.dma_start(dbg_m1[:].rearrange("(r t) -> r t", t=NT), m_dir[1][:])
                nc.sync.dma_start(dbg_loop[:].rearrange("(r t) -> r t", t=NT), loop[:])
                nc.sync.dma_start(dbg_inds[:].rearrange("(r t) -> r t", t=NT), indsn[:])

    nc.compile()
    return nc


def _get_program():
    if "nc" not in _CACHE:
        _CACHE["nc"] = _build_program()
    return _CACHE["nc"]


def _make_consts():
    if "consts" in _CACHE:
        return _CACHE["consts"]
    p = np.arange(128)
    c_indsn = (128 * np.arange(NT)[None, :] + p[:, None]).astype(np.float32)
    diag = (np.arange(16)[None, :] == (p % 16)[:, None])  # [128, 16]
    c_diagf = np.tile(diag, (1, NT)).astype(np.float32)
    consts = {"c_indsn": c_indsn, "c_diagf": c_diagf}
    _CACHE["consts"] = consts
    return consts


def _make_in_maps(descriptors0, descriptors1):
    consts = _make_consts()
    in_maps = []
    for c in range(B):
        a = np.ascontiguousarray(descriptors0[c].reshape(2, 128, N)).astype(
            ml_dtypes.bfloat16
        )
        bb = np.ascontiguousarray(descriptors1[c].reshape(2, 128, M)).astype(
            ml_dtypes.bfloat16
        )
        in_maps.append({"d0": a, "d1": bb, **consts})
    return in_maps


def kernel(descriptors0: np.ndarray, descriptors1: np.ndarray):
    from concourse.bass_utils import run_bass_kernel_spmd

    nc = _get_program()
    in_maps = _make_in_maps(descriptors0, descriptors1)
    res = run_bass_kernel_spmd(nc, in_maps, core_ids=list(range(B)))
    # outputs are written r-major ([128 partitions, NT tiles]); m = 128*t + r
    matches = np.stack([
        np.asarray(res.results[c]["matches"]).reshape(128, NT).T.reshape(-1)
        for c in range(B)
    ])
    scores = np.stack([
        np.asarray(res.results[c]["scores"]).reshape(128, NT).T.reshape(-1)
        for c in range(B)
    ])
    return matches.astype(np.int32), scores.astype(np.float32)


# revision 34
# speedup vs baseline: 1.6839x; 1.0123x over previous
"""Trainium2 Bass kernel for mutual-nearest-neighbor matching (Lowe ratio test).

Per-core layout: batch b=8 is sharded 1 batch element per NeuronCore (8 cores).
Each core computes, for its batch element:
  sim = d0^T @ d1          [n=4096, m=4096]   (bf16 matmuls, fp32 PSUM accum)
  top-2 + argmax along m  -> matches0 candidates + ratio mask + scores
  sim^T = d1^T @ d0        (second matmul direction)
  top-2 + argmax along n  -> matches1 candidates + ratio mask
  mutual check (fully local, via one small wrapped gather)
Outputs: matches int32 [4096], scores f32 [4096] per core; host stacks to [8, 4096].

v2 changes vs baseline:
  - PSUM halves are evicted by ACT to fp16 (10-bit mantissa) instead of bf16;
    4 low mantissa bits are reserved for fold-branch metadata (comb-16), which
    still leaves 6 value bits -- better precision than the bf16/3-bit baseline.
  - 4 fold levels (4096 -> 256) before Max8/MaxIndex, shrinking the 1x-rate
    top-8 scan from 512 to 256 columns.
  - The two full-width mantissa-cleanup TensorScalar passes can run on the
    (otherwise idle) GPSIMD engine instead of DVE (AND_ON_GPSIMD flag).
  - Direction-1 epilogue is chunked; each chunk's matches1 slice is bounced to
    DRAM and partition-broadcast while later tiles still compute, removing the
    ~65us serialized mutual-check tail.
  - Input DMAs are split so the first row-tile's matmuls start earlier.
"""

import sys

if "/opt/trn_rl_repo" not in sys.path:
    sys.path.insert(0, "/opt/trn_rl_repo")

import numpy as np
import ml_dtypes

B, D, N, M = 8, 256, 4096, 4096
NT = N // 128            # 32 row tiles per direction
HALF = M // 2            # 2048 columns per PSUM half-tile
NBANK = HALF // 512      # 4 matmul banks per half-tile
NSLOT = NT               # 32 row-tile slots per direction
RATIO2 = 0.8 * 0.8       # Lowe ratio threshold squared

FW4 = M // 16            # 256: width of the final fold array F4
MASK = 0xFFF0            # keep 6 fp16 mantissa bits, reserve 4 for branch bits
NCHUNK = 4               # dir-1 epilogue chunks (NSLOT must divide evenly)
AND_ON_GPSIMD = False    # GPSIMD lacks TENSOR_SCALAR support on TRN2 codegen

_CACHE: dict = {}


def _build_program(debug=False):
    import concourse.mybir as mybir
    import concourse.tile as tile
    from concourse import bacc

    dt = mybir.dt
    Alu = mybir.AluOpType

    nc = bacc.Bacc("TRN2", target_bir_lowering=False, debug=False)

    d0_dram = nc.dram_tensor("d0", [2, 128, N], dt.bfloat16, kind="ExternalInput")
    d1_dram = nc.dram_tensor("d1", [2, 128, M], dt.bfloat16, kind="ExternalInput")
    matches_dram = nc.dram_tensor("matches", [N], dt.int32, kind="ExternalOutput")
    scores_dram = nc.dram_tensor("scores", [N], dt.float32, kind="ExternalOutput")
    m1_bounce = nc.dram_tensor("m1_bounce", [M], dt.float32)  # internal
    c_indsn_dram = nc.dram_tensor("c_indsn", [128, NT], dt.float32, kind="ExternalInput")
    c_diagf_dram = nc.dram_tensor("c_diagf", [128, 16 * NT], dt.float32, kind="ExternalInput")
    if debug:
        dbg_m0 = nc.dram_tensor("dbg_m0", [N], dt.float32, kind="ExternalOutput")
        dbg_m1 = nc.dram_tensor("dbg_m1", [M], dt.float32, kind="ExternalOutput")
        dbg_loop = nc.dram_tensor("dbg_loop", [N], dt.float32, kind="ExternalOutput")
        dbg_inds = nc.dram_tensor("dbg_inds", [N], dt.float32, kind="ExternalOutput")

    with tile.TileContext(nc) as tc:
        with (
            tc.tile_pool(name="w", bufs=1) as wpool,
            tc.tile_pool(name="consts", bufs=1) as cpool,
            tc.tile_pool(name="acc", bufs=1) as apool,
            tc.tile_pool(name="x", bufs=4) as xpool,
            tc.tile_pool(name="f", bufs=3) as fpool,
            tc.tile_pool(name="psum", bufs=2, space="PSUM") as ppool,
        ):
            # ---- load descriptors (already bf16, k-major [2, 128, N]) ----
            # d1 first (tile-0 matmuls need all of d1); d0 split so its first
            # columns arrive early.
            d0_sb = [wpool.tile([128, N], dt.bfloat16, name=f"d0_{k}") for k in range(2)]
            d1_sb = [wpool.tile([128, M], dt.bfloat16, name=f"d1_{k}") for k in range(2)]
            for k in range(2):
                nc.sync.dma_start(d1_sb[k][:], d1_dram[k])
            D0SPLIT = 512
            for k in range(2):
                nc.sync.dma_start(d0_sb[k][:, :D0SPLIT], d0_dram[k][:, :D0SPLIT])
            for k in range(2):
                nc.sync.dma_start(d0_sb[k][:, D0SPLIT:], d0_dram[k][:, D0SPLIT:])

            # ---- constants (host-provided) ----
            indsn = cpool.tile([128, NT], dt.float32, name="indsn")
            nc.sync.dma_start(indsn[:], c_indsn_dram[:])
            diag_f = cpool.tile([128, 16 * NT], dt.float32, name="diag_f")
            nc.sync.dma_start(diag_f[:], c_diagf_dram[:])

            # prime the GPSIMD ucode library (indirect_copy) while input DMAs
            # run, so the IRAM load is off the critical path at the end.
            prime = cpool.tile([128, 16], dt.float32, name="prime")
            zidx = cpool.tile([128, 1], dt.uint16, name="zidx")
            nc.vector.memset(zidx[:], 0)
            nc.gpsimd.indirect_copy(prime[:, :], diag_f[:, :16], zidx[:], True)

            # ---- per-direction accumulators ----
            t8_acc = [apool.tile([128, NSLOT * 8], dt.float16, name=f"t8_{d}") for d in range(2)]
            pi_acc = [apool.tile([128, NSLOT * 8], dt.uint16, name=f"pi_{d}") for d in range(2)]

            m_dir = [apool.tile([128, NT], dt.float32, name=f"mdir_{d}") for d in range(2)]
            scores0 = apool.tile([128, NT], dt.float32, name="scores0")

            # epilogue working tiles (slices written per chunk)
            t8c = [apool.tile([128, NSLOT * 8], dt.uint16, name=f"t8c_{d}") for d in range(2)]
            v1g = [apool.tile([128, NT], dt.float32, name=f"v1g_{d}") for d in range(2)]
            v2g = [apool.tile([128, NT], dt.float32, name=f"v2g_{d}") for d in range(2)]
            pf = [apool.tile([128, NSLOT], dt.float32, name=f"pf_{d}") for d in range(2)]
            bu = [[apool.tile([128, NSLOT], dt.uint16, name=f"b{i}u_{d}") for i in range(4)]
                  for d in range(2)]
            bf = [[apool.tile([128, NSLOT], dt.float32, name=f"b{i}f_{d}") for i in range(4)]
                  for d in range(2)]
            mst = [apool.tile([128, NSLOT], dt.float32, name=f"mst_{d}") for d in range(2)]
            acc1 = [apool.tile([128, NT], dt.float32, name=f"acc1_{d}") for d in range(2)]
            maskf = [apool.tile([128, NT], dt.uint8, name=f"maskf_{d}") for d in range(2)]

            m1_rep = apool.tile([128, M], dt.float32, name="m1_rep")

            def epilogue(dire, s0, s1):
                """Ratio test + match-index decode for tile slots [s0, s1)."""
                ns = s1 - s0
                t8a, pia = t8_acc[dire], pi_acc[dire]
                # strip the embedded index bits from the stored top-8 values
                nc.vector.tensor_scalar(
                    t8c[dire][:, 8 * s0 : 8 * s1].bitcast(dt.uint16),
                    t8a[:, 8 * s0 : 8 * s1].bitcast(dt.uint16), MASK, None,
                    op0=Alu.bitwise_and,
                )
                A3 = t8c[dire][:, 8 * s0 : 8 * s1].bitcast(dt.float16).rearrange(
                    "p (g e) -> p g e", e=8)
                A3u = t8a[:, 8 * s0 : 8 * s1].bitcast(dt.uint16).rearrange(
                    "p (g e) -> p g e", e=8)
                P3 = pia[:, 8 * s0 : 8 * s1].rearrange("p (g e) -> p g e", e=8)

                sl = (slice(None), slice(s0, s1))
                nc.vector.tensor_copy(v1g[dire][sl], A3[:, :, 0])
                nc.vector.tensor_copy(v2g[dire][sl], A3[:, :, 1])
                nc.vector.tensor_copy(pf[dire][sl], P3[:, :, 0])

                # decode the winner's branch bits: bit0 (X level, weight 2048),
                # bit1 (F1 level, raw 2 -> weight 1024), bit2 (F2 level, raw 4
                # -> weight 512), bit3 (F3 level, raw 8 -> weight 256)
                for i, rawbit in enumerate([1, 2, 4, 8]):
                    nc.vector.tensor_scalar(
                        bu[dire][i][sl], A3u[:, :, 0], rawbit, None,
                        op0=Alu.bitwise_and,
                    )
                    nc.vector.tensor_copy(bf[dire][i][sl], bu[dire][i][sl])

                # absolute column: m = p + 2048*b0 + 1024*(b1/2) + 512*(b2/4)
                #                        + 256*(b3/8)
                nc.vector.scalar_tensor_tensor(
                    mst[dire][sl], bf[dire][0][sl], 2048.0, pf[dire][sl],
                    op0=Alu.mult, op1=Alu.add,
                )
                for i, w in [(1, 512.0), (2, 128.0), (3, 32.0)]:
                    nc.vector.scalar_tensor_tensor(
                        mst[dire][sl], bf[dire][i][sl], w, mst[dire][sl],
                        op0=Alu.mult, op1=Alu.add,
                    )

                # ratio test: dist1 <= r^2*dist2  <=>  v1 - r^2*v2 >= 1 - r^2
                nc.vector.scalar_tensor_tensor(
                    acc1[dire][sl], v2g[dire][sl], -RATIO2, v1g[dire][sl],
                    op0=Alu.mult, op1=Alu.add,
                )
                nc.vector.tensor_scalar(
                    maskf[dire][sl], acc1[dire][sl], 1.0 - RATIO2, None,
                    op0=Alu.is_ge,
                )
                if dire == 0:
                    sc = apool.tile([128, NT], dt.float32, name="sc")
                    nc.vector.tensor_scalar(
                        sc[sl], v1g[0][sl], 0.5, 0.5, op0=Alu.mult, op1=Alu.add
                    )
                    nc.vector.tensor_mul(scores0[sl], sc[sl], maskf[0][sl])
                nc.vector.memset(m_dir[dire][sl], -1.0)
                nc.vector.copy_predicated(m_dir[dire][sl], maskf[dire][sl], mst[dire][sl])

            and_eng = nc.gpsimd if AND_ON_GPSIMD else nc.vector

            for dire in range(2):
                lhs = d0_sb if dire == 0 else d1_sb
                rhs = d1_sb if dire == 0 else d0_sb
                t8a, pia = t8_acc[dire], pi_acc[dire]
                chunk = NSLOT // NCHUNK

                for t in range(NT):
                    s = t
                    X = xpool.tile([128, M], dt.float16, name=f"X_{dire}_{s}", tag="X")
                    for h in range(2):
                        P = ppool.tile([128, HALF], dt.float32, name=f"P_{dire}_{s}_{h}", tag="P")
                        for k in range(2):
                            for b in range(NBANK):
                                nc.tensor.matmul(
                                    P[:, 512 * b : 512 * (b + 1)],
                                    lhs[k][:, 128 * t : 128 * (t + 1)],
                                    rhs[k][:, HALF * h + 512 * b : HALF * h + 512 * (b + 1)],
                                    start=(k == 0),
                                    stop=(k == 1),
                                )
                        nc.scalar.copy(X[:, HALF * h : HALF * (h + 1)], P[:])
                    # bit-packed folds: truncate the 4 low mantissa bits and OR a
                    # fold-branch bit into each fold's right operand.  The fold
                    # winner then carries its own comb-branch bits.
                    Xu = X[:].bitcast(dt.uint16)
                    XL = fpool.tile([128, M // 2], dt.float16, name=f"XL_{dire}_{s}", tag="XL")
                    and_eng.tensor_scalar(
                        XL[:].bitcast(dt.uint16), Xu[:, : M // 2], MASK, None,
                        op0=Alu.bitwise_and,
                    )
                    XR = fpool.tile([128, M // 2], dt.float16, name=f"XR_{dire}_{s}", tag="XR")
                    and_eng.tensor_scalar(
                        XR[:].bitcast(dt.uint16), Xu[:, M // 2 :], MASK, 1,
                        op0=Alu.bitwise_and, op1=Alu.bitwise_or,
                    )
                    F1 = fpool.tile([128, M // 2], dt.float16, name=f"F1_{dire}_{s}", tag="F1")
                    nc.vector.tensor_max(F1[:], XL[:], XR[:])
                    FR2 = fpool.tile([128, M // 4], dt.float16, name=f"FR2_{dire}_{s}", tag="FR2")
                    nc.vector.tensor_scalar(
                        FR2[:].bitcast(dt.uint16), F1[:].bitcast(dt.uint16)[:, M // 4 :], 2, None,
                        op0=Alu.bitwise_or,
                    )
                    F2 = fpool.tile([128, M // 4], dt.float16, name=f"F2_{dire}_{s}", tag="F2")
                    nc.vector.tensor_max(F2[:], F1[:, : M // 4], FR2[:])
                    FR3 = fpool.tile([128, M // 8], dt.float16, name=f"FR3_{dire}_{s}", tag="FR3")
                    nc.vector.tensor_scalar(
                        FR3[:].bitcast(dt.uint16), F2[:].bitcast(dt.uint16)[:, M // 8 :], 4, None,
                        op0=Alu.bitwise_or,
                    )
                    F3 = fpool.tile([128, M // 8], dt.float16, name=f"F3_{dire}_{s}", tag="F3")
                    nc.vector.tensor_max(F3[:], F2[:, : M // 8], FR3[:])
                    FR4 = fpool.tile([128, FW4], dt.float16, name=f"FR4_{dire}_{s}", tag="FR4")
                    nc.vector.tensor_scalar(
                        FR4[:].bitcast(dt.uint16), F3[:].bitcast(dt.uint16)[:, FW4:], 8, None,
                        op0=Alu.bitwise_or,
                    )
                    F4 = fpool.tile([128, FW4], dt.float16, name=f"F4_{dire}_{s}", tag="F4")
                    nc.vector.tensor_max(F4[:], F3[:, :FW4], FR4[:])

                    t8_slot = t8a[:, 8 * s : 8 * s + 8]
                    pi_slot = pia[:, 8 * s : 8 * s + 8]
                    nc.vector.max(t8_slot, F4[:])
                    nc.vector.max_index(pi_slot, t8_slot, F4[:])

                    # dir-1: as soon as a chunk of tile slots is complete, run
                    # its epilogue and ship its matches1 slice (r-major DRAM
                    # layout -> 32B-contiguous runs instead of 4B scatter) so
                    # only the final broadcast remains at the end.
                    if dire == 1 and (t + 1) % chunk == 0:
                        c = (t + 1) // chunk - 1
                        s0, s1 = c * chunk, (c + 1) * chunk
                        epilogue(1, s0, s1)
                        # chunk-major bounce: chunk c owns the contiguous
                        # DRAM range [1024c, 1024c+1024) as [r, t'] (flat
                        # q = 1024c + 8r + t'), so both the write and the
                        # broadcast re-read are contiguous; the broadcast
                        # overlaps the remaining tiles.
                        mlo, mhi = 128 * chunk * c, 128 * chunk * (c + 1)
                        nc.sync.dma_start(
                            m1_bounce[mlo:mhi].rearrange("(r u) -> r u", u=chunk),
                            m_dir[1][:, s0:s1],
                        )
                        nc.sync.dma_start(
                            m1_rep[:, mlo:mhi],
                            m1_bounce[mlo:mhi][None, :].partition_broadcast(128),
                        )

                if dire == 0:
                    epilogue(0, 0, NSLOT)
                    # scores are final after the dir-0 epilogue; ship them now
                    # (r-major DRAM layout; host transposes)
                    nc.sync.dma_start(
                        scores_dram[:].rearrange("(r t) -> r t", t=NT), scores0[:]
                    )
                    # gather indices for the mutual check, permuted into the
                    # chunk-major layout of m1_rep: m value v (= 128t + r) ->
                    # q = 1024*(t>>3) + 8*r + (t&7)
                    safe = apool.tile([128, NT], dt.float32, name="safe")
                    nc.vector.tensor_scalar_max(safe[:], m_dir[0][:], 0.0)
                    safe16 = apool.tile([128, NT], dt.uint16, name="safe16")
                    nc.vector.tensor_copy(safe16[:], safe[:])
                    cq = apool.tile([128, NT], dt.uint16, name="cq")
                    nc.vector.tensor_scalar(
                        cq[:], safe16[:], 10, 10,
                        op0=Alu.logical_shift_right, op1=Alu.logical_shift_left,
                    )
                    rq = apool.tile([128, NT], dt.uint16, name="rq")
                    nc.vector.tensor_scalar(
                        rq[:], safe16[:], 127, 3,
                        op0=Alu.bitwise_and, op1=Alu.logical_shift_left,
                    )
                    tq = apool.tile([128, NT], dt.uint16, name="tq")
                    nc.vector.tensor_scalar(
                        tq[:], safe16[:], 7, 7,
                        op0=Alu.logical_shift_right, op1=Alu.bitwise_and,
                    )
                    idx16 = apool.tile([128, NT], dt.uint16, name="idx16")
                    nc.vector.tensor_tensor(idx16[:], cq[:], rq[:], op=Alu.bitwise_or)
                    nc.vector.tensor_tensor(idx16[:], idx16[:], tq[:], op=Alu.bitwise_or)

            # ---- mutual check ----
            # (m1_rep was broadcast per chunk above)
            gm = apool.tile([128, 16 * NT], dt.float32, name="gm")
            nc.gpsimd.indirect_copy(gm[:], m1_rep[:], idx16[:], True)
            gmp = apool.tile([128, 16 * NT], dt.float32, name="gmp")
            nc.vector.tensor_mul(gmp[:], gm[:], diag_f[:])
            loop = apool.tile([128, NT], dt.float32, name="loop")
            nc.vector.tensor_reduce(
                loop[:],
                gmp[:].rearrange("p (j u) -> p j u", u=16),
                axis=mybir.AxisListType.X,
                op=Alu.add,
            )

            g1 = apool.tile([128, NT], dt.uint8, name="g1")
            nc.vector.tensor_scalar(g1[:], m_dir[0][:], -0.5, None, op0=Alu.is_gt)
            g2 = apool.tile([128, NT], dt.uint8, name="g2")
            nc.vector.tensor_tensor(g2[:], indsn[:], loop[:], op=Alu.is_equal)
            okm = apool.tile([128, NT], dt.uint8, name="okm")
            nc.vector.tensor_mul(okm[:], g1[:], g2[:])

            mfin = apool.tile([128, NT], dt.float32, name="mfin")
            nc.vector.memset(mfin[:], -1.0)
            nc.vector.copy_predicated(mfin[:], okm[:], m_dir[0][:])
            mi32 = apool.tile([128, NT], dt.int32, name="mi32")
            nc.vector.tensor_copy(mi32[:], mfin[:])

            nc.sync.dma_start(matches_dram[:].rearrange("(r t) -> r t", t=NT), mi32[:])
            if debug:
                nc.sync.dma_start(dbg_m0[:].rearrange("(r t) -> r t", t=NT), m_dir[0][:])
                nc.sync.dma_start(dbg_m1[:].rearrange("(r t) -> r t", t=NT), m_dir[1][:])
                nc.sync.dma_start(dbg_loop[:].rearrange("(r t) -> r t", t=NT), loop[:])
                nc.sync.dma_start(dbg_inds[:].rearrange("(r t) -> r t", t=NT), indsn[:])

    nc.compile()
    return nc


def _get_program():
    if "nc" not in _CACHE:
        _CACHE["nc"] = _build_program()
    return _CACHE["nc"]


def _make_consts():
    if "consts" in _CACHE:
        return _CACHE["consts"]
    p = np.arange(128)
    c_indsn = (128 * np.arange(NT)[None, :] + p[:, None]).astype(np.float32)
    diag = (np.arange(16)[None, :] == (p % 16)[:, None])  # [128, 16]
    c_diagf = np.tile(diag, (1, NT)).astype(np.float32)
    consts = {"c_indsn": c_indsn, "c_diagf": c_diagf}
    _CACHE["consts"] = consts
    return consts


def _make_in_maps(descriptors0, descriptors1):
    consts = _make_consts()
    in_maps = []
    for c in range(B):
        a = np.ascontiguousarray(descriptors0[c].reshape(2, 128, N)).astype(
            ml_dtypes.bfloat16
        )
        bb = np.ascontiguousarray(descriptors1[c].reshape(2, 128, M)).astype(
            ml_dtypes.bfloat16
        )
        in_maps.append({"d0": a, "d1": bb, **consts})
    return in_maps


def kernel(descriptors0: np.ndarray, descriptors1: np.ndarray):
    from concourse.bass_utils import run_bass_kernel_spmd

    nc = _get_program()
    in_maps = _make_in_maps(descriptors0, descriptors1)
    res = run_bass_kernel_spmd(nc, in_maps, core_ids=list(range(B)))
    # outputs are written r-major ([128 partitions, NT tiles]); m = 128*t + r
    matches = np.stack([
        np.asarray(res.results[c]["matches"]).reshape(128, NT).T.reshape(-1)
        for c in range(B)
    ])
    scores = np.stack([
        np.asarray(res.results[c]["scores"]).reshape(128, NT).T.reshape(-1)
        for c in range(B)
    ])
    return matches.astype(np.int32), scores.astype(np.float32)
